# revision 39
# baseline (speedup 1.0000x reference)
"""Causal self-attention (B=4, T=2048, C=1024, H=16, D=64) on 8 trn2 cores.

Sharding: core i handles batch b = i//2 and head-group g = i%2 (8 of 16
heads), tensor-parallel over c_attn columns / c_proj rows. Each core
computes qkv for its heads, causal attention, and a partial projection
(its 512 rows of w_proj); the host sums the two partials per batch and
adds b_proj.

v2: qkv-gen uses fp8e4 DoubleRow matmuls (0.5 cyc/col, 2 K-tiles per
instruction) with host-precomputed hi/lo fp8 splits of x (scale 1) and
w (scale 64 qk / 32 v); three passes (wh*xh + wl*xh + wh*xl) recover
bf16-grade accuracy at 6/8 the bf16 matmul cost. The attention S matmul
is one fp8 DoubleRow op per strip: the stationary packs (k_hi, k_lo) as
the two K-tiles (k exact via the tile-sum), the moving operand
broadcasts a single-fp8 q; the k path skips its x_lo correction pass
(measured output rel err 1.63e-2; gate 2e-2).
k biases ride in the k_hi cast (zero for these inputs). es/vaug/yT/proj run bf16 (exp scale
0.125/4096, no f32r widening), output is written bf16 and summed on the
host. psum->sbuf q8/k casts sit on DVE to keep ACT (exp-bound) clear.

v3: the A.V matmul is reoriented: out[128 tok, D+1] = es(stationary)^T
x vaug(moving) so the modeled cost is 65 cols/instr instead of the
q-strip width (PE A.V 58us -> 29us). Softmax denominators land
per-token-partition, so normalization is a [P,4] reciprocal + one
broadcast tensor_tensor per 4-chunk group (replaces the [1,512]
reciprocal + gpsimd partition_broadcast + wide multiply). y[tok, feat]
is flipped to yT[feat, tok] for the projection with XBAR
dma_start_transpose (DMA-resource only, no engine time). Out-staging
psum->sbuf copies move off ACT (exp is the binding engine) onto
DVE/gpsimd.
"""

import sys

sys.path.insert(0, "/opt/trn_rl_repo")

from collections import deque
from contextlib import ExitStack

import ml_dtypes
import numpy as np

import concourse.bass as bass
import concourse.mybir as mybir
import concourse.tile as tile
from concourse import bacc
from concourse import bass_utils

f32 = mybir.dt.float32
f32r = mybir.dt.float32r
bf16 = mybir.dt.bfloat16
fp8 = mybir.dt.float8e4
EXP = mybir.ActivationFunctionType.Exp
MUL = mybir.AluOpType.mult
ADD = mybir.AluOpType.add
DR = mybir.MatmulPerfMode.DoubleRow

B, T, C, H, D = 4, 2048, 1024, 16, 64
HL = H // 2          # 8 heads per core
CL = HL * D          # 512 local feature width
P = 128
KC = C // P          # 8 contraction chunks over C
NJC = T // P         # 16 token chunks of 128
NTC = T // 512       # 4 t-chunks of 512 in phase A
JQK = 2 * CL // P    # 8 qk feature chunks of 128

SW = 64.0            # host scale on w_qk (psum = 64*qk)
SV = 32.0            # host scale on w_v (v path scaled by 32)
EXP_SCALE = 0.125 / (SW * SW)

# position of v_chunk[j] in the filler deque (q-chunk leftovers interleave)
POS_V = [0, 1, 3, 4, 6, 7, 9, 10, 12, 13, 14, 15, 16, 17, 18, 19]

# Schraudolph-style exp on DVE: bf16 bits of exp(psum*EXP_SCALE) ~=
# trunc(psum*SCH_A + SCH_B) as int16 (rel err ~1.8% rms, zero-mean).
# Offloads full-width strips from the ACT engine (the busiest engine).
SCH_A = (2.0**23 / np.log(2.0)) / 65536.0 * EXP_SCALE
SCH_B = 16249.0
SCH_JC = (1, 3, 5, 6)   # which pp=1 full-width strips go to DVE


def build_body(tc, aps):
    nc = tc.nc
    bqk, bv, wp, masks, outp = (
        aps["bqk"], aps["bv"], aps["wp"], aps["masks"], aps["outp"],
    )

    with ExitStack() as ctx:
        const = ctx.enter_context(tc.tile_pool(name="const", bufs=1))
        qkT_pool = ctx.enter_context(tc.tile_pool(name="qkT", bufs=1))
        vaug_pool = ctx.enter_context(tc.tile_pool(name="vaug", bufs=1))
        yT_pool = ctx.enter_context(tc.tile_pool(name="yT", bufs=1))

        # const tiles; their DMAs are emitted after the critical xh/xl/wq
        # ones (the DMA resource is serial and these aren't needed early)
        masks_sb = const.tile([P, P], bf16)
        bqk_sb = const.tile([P, JQK], f32)
        bv_rep = const.tile([P, CL], f32)

        def const_dmas():
            nc.sync.dma_start(bqk_sb[:], bqk.rearrange("(j p) -> p j", p=P))
            nc.sync.dma_start(masks_sb[:], masks[:])
            nc.sync.dma_start(bv_rep[:], bv[None, :].to_broadcast([P, CL]))

        q8 = qkT_pool.tile([P, JQK // 2, T], fp8, name="q8")
        khl = qkT_pool.tile([P, JQK // 2, 2, T], fp8, name="khl")
        vaug = vaug_pool.tile([P, NJC, HL, D + 1], bf16)
        # ones column holds the v-path scale so y = num/(SV*sum_es) * SV
        nc.vector.memset(
            vaug[:, :, :, D : D + 1].bitcast(mybir.dt.uint16), 0x4200
        )
        # y in [token-partition, feature] layout; transposed to yT by DMA
        y_sb = yT_pool.tile([P, NJC, HL, D], bf16, name="y_sb")
        # all strips of one (h, pp) group stay live (A.V is qc-major so
        # each psum accumulation group opens/closes sequentially per bank);
        # ping-pong across groups so group g+1's exps don't wait on group
        # g's last A.V burst (which reads every strip slot)
        es_ab = [
            yT_pool.tile([P, NJC, 1024], bf16, name=f"es_all{i}")
            for i in range(2)
        ]
        _gctr = {"g": 0}

        # ---------------- Phase A/B interleaved (same skeleton as v1).
        def qk_chunk_thunks(j, xh_sb, xl_sb):
            wq_box = {}

            def preload():
                wqh_t = wq_pool.tile([P, KC, P], fp8, tag="wqh", name=f"wqh{j}")
                nc.sync.dma_start(wqh_t[:], aps["wqk_hi"][j])
                wql_t = wq_pool.tile([P, KC, P], fp8, tag="wql", name=f"wql{j}")
                nc.sync.dma_start(wql_t[:], aps["wqk_lo"][j])
                wq_box["t"] = (wqh_t, wql_t)

            def load(tci):
                if tci == 0 and "t" not in wq_box:
                    preload()
                wqh_t, wql_t = wq_box["t"]
                is_k = j >= JQK // 2
                ps = psA.tile([P, 512], f32, tag="a")
                cols = slice(tci * 512, tci * 512 + 512)
                # k drops the x_lo correction pass: its ~2.5% error adds in
                # quadrature to the S-stage q8 error (1.19e-2 -> 1.63e-2
                # measured, gate 2e-2) and saves 4 DR matmuls per group.
                # (dropping it on q too was measured at 2.1e-2 — over gate)
                passes = ((wqh_t, xh_sb), (wql_t, xh_sb)) if is_k else (
                    (wqh_t, xh_sb), (wql_t, xh_sb), (wqh_t, xl_sb))
                last = KC // 2 * len(passes) - 1
                n = 0
                # pass-major so the xh-only passes can run before xl lands
                for wt, xt in passes:
                    for i in range(KC // 2):
                        nc.tensor.matmul(
                            ps[:], wt[:, 2 * i : 2 * i + 2, :],
                            xt[:, 2 * i : 2 * i + 2, cols],
                            start=(n == 0),
                            stop=(n == last),
                            perf_mode=DR,
                        )
                        n += 1
                if is_k:
                    kh = khl[:, j - JQK // 2, 0, cols]
                    nc.vector.tensor_scalar(
                        kh, ps[:], bqk_sb[:, j : j + 1], None, ADD
                    )
                    nc.vector.tensor_tensor(
                        khl[:, j - JQK // 2, 1, cols], ps[:], kh,
                        mybir.AluOpType.subtract,
                    )
                else:
                    nc.vector.tensor_scalar(
                        q8[:, j, cols], ps[:], bqk_sb[:, j : j + 1], None, ADD
                    )

            thunks = [lambda tci=tci: load(tci) for tci in range(NTC)]
            return thunks, preload

        def v_chunk_thunks(xh_sb, xl_sb, wvh_sb, wvl_sb):
            def vchunk(tc128):
                ps = psA.tile([P, 512], f32, tag="a")
                cols = slice(tc128 * P, (tc128 + 1) * P)
                n = 0
                for i in range(KC // 2):
                    for xt, wt in (
                        (xh_sb, wvh_sb), (xh_sb, wvl_sb), (xl_sb, wvh_sb),
                    ):
                        nc.tensor.matmul(
                            ps[:], xt[:, 2 * i : 2 * i + 2, cols],
                            wt[:, 2 * i : 2 * i + 2, :],
                            start=(n == 0), stop=(n == 3 * KC // 2 - 1),
                            perf_mode=DR,
                        )
                        n += 1
                nc.vector.tensor_tensor(
                    vaug[:, tc128, :, 0:D],
                    ps[:].rearrange("p (h d) -> p h d", h=HL),
                    bv_rep[:].rearrange("p (h d) -> p h d", h=HL), ADD,
                )

            return [lambda t=t: vchunk(t) for t in range(NJC)]

        def emit_b_pass(h, pp, pace, post_norm=None):
            pr, half = h // 2, h % 2
            base = half * 64

            es_all = es_ab[_gctr["g"] % 2]
            _gctr["g"] += 1
            lo = pp * 1024
            strip_start = {}
            box = {}

            def burst(qc):
                # A.V for q-chunk qc: one psum accumulation group open at a
                # time per bank (zero-region rule). Runs 2 strips late via
                # burst_q so next strips' S matmuls outrank it.
                need(POS_V[qc] + 1)
                s = qc - 8 * pp
                if s % 4 == 0:
                    box["grp"] = psY.tile(
                        [P, 4, D + 1], f32, tag="yt",
                        name=f"yt{h}_{pp}_{s // 4}",
                    )
                grp = box["grp"]
                slot = s % 4
                for j2 in range(qc + 1):
                    st2 = strip_start[j2]
                    nc.tensor.matmul(
                        grp[:, slot, :],
                        es_all[:, j2, qc * P - st2 : qc * P - st2 + P],
                        vaug[:, j2, h, :],
                        start=(j2 == 0), stop=(j2 == qc),
                    )
                if s % 4 == 3:
                    rec4 = nrm_pool.tile([P, 4], f32, tag="rec")
                    nc.vector.reciprocal(
                        rec4[:, :, None], grp[:, :, D : D + 1]
                    )
                    nc.vector.tensor_tensor(
                        y_sb[:, qc - 3 : qc + 1, h, :],
                        grp[:, :, 0:D],
                        rec4[:, :, None].to_broadcast([P, 4, D]),
                        MUL,
                    )
                    if post_norm is not None:
                        post_norm(qc)

            for jc in range(8 * pp + 8):
                need(pace(jc))
                diag = jc * P >= lo
                start = max(jc * P, lo)
                strip_start[jc] = start
                ps_s = psS.tile([P, 1024], f32, tag="s")
                for icp in range(2):
                    ic = 2 * pp + icp
                    if (ic + 1) * 512 <= start:
                        continue
                    c0 = max(start, ic * 512)
                    nc.tensor.matmul(
                        ps_s[:, c0 - lo : (icp + 1) * 512],
                        khl[base : base + 64, pr, :, jc * P : (jc + 1) * P],
                        q8[base : base + 64, pr : pr + 1, c0 : (ic + 1) * 512]
                        .to_broadcast([64, 2, (ic + 1) * 512 - c0]),
                        start=True, stop=True,
                        perf_mode=DR,
                    )
                w = lo + 1024 - start
                if h == 0 and pp == 0 and jc < 4:
                    # startup: exp each 512-col half as its S matmul lands so
                    # the first exps don't wait for the q8 tci-1 chunk
                    for icp in range(2):
                        c0 = max(start, icp * 512)
                        nc.scalar.activation(
                            es_all[:, jc, c0 - start : (icp + 1) * 512 - start],
                            ps_s[:, c0 - lo : (icp + 1) * 512],
                            EXP, scale=EXP_SCALE,
                        )
                elif pp == 1 and jc in SCH_JC:
                    # DVE exp (Schraudolph int16/bf16 bit trick) to offload
                    # the saturated ACT engine on no-diag full-width strips
                    nc.vector.tensor_scalar(
                        es_all[:, jc, :w].bitcast(mybir.dt.int16),
                        ps_s[:, 0:1024], SCH_A, SCH_B, MUL, ADD,
                    )
                else:
                    nc.scalar.activation(
                        es_all[:, jc, :w], ps_s[:, start - lo : 1024],
                        EXP, scale=EXP_SCALE,
                    )
                if diag:
                    nc.vector.tensor_tensor(
                        es_all[:, jc, 0:P], es_all[:, jc, 0:P], masks_sb[:], MUL
                    )
                if jc >= 8 * pp:
                    burst_q.append(lambda qc=jc: burst(qc))
                pump_bursts(2)

        def pair_tp(hp, qlo, qhi):
            # yT[feat, tok] <- y_sb[tok, feat] for head pair hp via XBAR
            for qc in range(qlo, qhi):
                nc.sync.dma_start_transpose(
                    yT[:, hp, qc * P : (qc + 1) * P],
                    y_sb[:, qc, 2 * hp : 2 * hp + 2, :],
                )

        yT = yT_pool.tile([P, CL // P, T], bf16)
        with ExitStack() as actx:
            xt_pool = actx.enter_context(tc.tile_pool(name="xt", bufs=1))
            wq_pool = actx.enter_context(tc.tile_pool(name="wq", bufs=2))
            psA = actx.enter_context(tc.tile_pool(name="psA", bufs=2, space="PSUM"))

            filler = deque()
            _dr = {"done": 0}

            def need(k):
                while filler and _dr["done"] < k:
                    filler.popleft()()
                    _dr["done"] += 1

            burst_q = deque()

            def pump_bursts(keep=2):
                while len(burst_q) > keep:
                    burst_q.popleft()()

            def flush_bursts():
                pump_bursts(0)

            with tc.tile_pool(name="wv", bufs=1) as wv_pool:
                xh_sb = xt_pool.tile([P, KC, T], fp8, name="xh")
                xl_sb = xt_pool.tile([P, KC, T], fp8, name="xl")
                xh_r = aps["x_hi"].rearrange("(k p) t -> p k t", p=P)
                xl_r = aps["x_lo"].rearrange("(k p) t -> p k t", p=P)

                def xq_dma(q):
                    cols = slice(q * 512, (q + 1) * 512)
                    nc.sync.dma_start(xh_sb[:, :, cols], xh_r[:, :, cols])
                    nc.sync.dma_start(xl_sb[:, :, cols], xl_r[:, :, cols])

                wvh_sb = wv_pool.tile([P, KC, CL], fp8, name="wvh")
                wvl_sb = wv_pool.tile([P, KC, CL], fp8, name="wvl")

                # warm the PE p-state clock so the first real matmuls run at
                # full speed (the cost model ramps over 3us of continuous
                # execution; PE would otherwise sit cold during the input DMAs)
                nc.vector.memset(es_ab[0][:, 0, 0:512].bitcast(mybir.dt.uint16), 0)
                warm_ps = psA.tile([P, 512], f32, tag="a", name="warm")
                for _ in range(14):
                    nc.tensor.matmul(
                        warm_ps[:], es_ab[0][:, 0, 0:P], es_ab[0][:, 0, 0:512],
                        start=True, stop=True,
                    )

                xq_dma(0)
                q0, pre0 = qk_chunk_thunks(0, xh_sb, xl_sb)
                q4, pre4 = qk_chunk_thunks(4, xh_sb, xl_sb)
                pre0()
                pre4()
                const_dmas()
                q0[0]()
                q4[0]()
                xq_dma(1)
                q0[1]()
                q4[1]()
                nc.sync.dma_start(
                    wvh_sb[:], aps["wv_hi"].rearrange("(k p) n -> p k n", p=P)
                )
                nc.sync.dma_start(
                    wvl_sb[:], aps["wv_lo"].rearrange("(k p) n -> p k n", p=P)
                )
                xq_dma(2)
                xq_dma(3)
                # tci 2,3 of j0/j4 drip through the filler so (0,0)'s strips
                # overlap them instead of serializing the whole eager prefix
                vs = v_chunk_thunks(xh_sb, xl_sb, wvh_sb, wvl_sb)
                filler.extend([
                    vs[0], vs[1], q0[2], vs[2], vs[3], q4[2],
                    vs[4], vs[5], q0[3], vs[6], vs[7], q4[3],
                ])
                filler.extend(vs[8:])
                for j in (1, 5):
                    filler.extend(qk_chunk_thunks(j, xh_sb, xl_sb)[0])

                with ExitStack() as bctx:
                    nrm_pool = bctx.enter_context(tc.tile_pool(name="nrm", bufs=1))
                    ostg = bctx.enter_context(tc.tile_pool(name="ostg", bufs=4))
                    wp_pool = bctx.enter_context(tc.tile_pool(name="wp", bufs=1))
                    psS = bctx.enter_context(
                        tc.tile_pool(name="psS", bufs=2, space="PSUM")
                    )
                    psY = bctx.enter_context(
                        tc.tile_pool(name="psY", bufs=2, space="PSUM")
                    )

                    def c_tile(tcb, oc):
                        ps = psA.tile([P, 512], f32, tag="a")
                        for li in range(4):
                            nc.tensor.matmul(
                                ps[:],
                                yT[:, li, tcb * P : (tcb + 1) * P],
                                wp_sb[:, li, oc * 512 : oc * 512 + 512],
                                start=(li == 0), stop=(li == 3),
                            )
                        ot = ostg.tile([P, 512], bf16, tag="o")
                        if tcb >= 8 and (tcb + oc) % 2 == 0:
                            # tail only: ACT is idle there; alternating with
                            # DVE pipelines the copy stage two wide
                            nc.scalar.copy(ot[:], ps[:])
                        else:
                            nc.vector.tensor_copy(ot[:], ps[:])
                        nc.sync.dma_start(
                            outp[tcb * P : (tcb + 1) * P,
                                 oc * 512 : oc * 512 + 512],
                            ot[:],
                        )

                    def pp1_pace(h):
                        return lambda jc: 44 + min(16, (h - 2) * 3 + jc // 6)

                    emit_b_pass(0, 0, lambda jc: 2 + jc)
                    for j in (2, 6, 3, 7):
                        filler.extend(qk_chunk_thunks(j, xh_sb, xl_sb)[0])
                    emit_b_pass(0, 1, lambda jc: 9 if jc < 10 else jc + 3)
                    emit_b_pass(1, 0, lambda jc: 20 + jc // 2)
                    emit_b_pass(1, 1, lambda jc: min(28, 24 + jc // 3))
                    # each pair_tp is placed 1+ passes after the last writer
                    # pass so its bursts have drained through burst_q without
                    # a flush (a flush lumps A.V work ahead of the next S's)
                    pair_tp(0, 0, 8)
                    emit_b_pass(2, 0, lambda jc: 28 + jc // 2)
                    pair_tp(0, 8, NJC)
                    emit_b_pass(3, 0, lambda jc: 32 + (jc + 1) // 2)
                    emit_b_pass(4, 0, lambda jc: 36 + jc // 2)
                    pair_tp(1, 0, 8)
                    emit_b_pass(5, 0, lambda jc: 40 + (jc + 1) // 2)
                    emit_b_pass(6, 0, lambda jc: 44)
                    wp_sb = wp_pool.tile([P, 4, C], bf16, name="wp_sb")
                    nc.sync.dma_start(
                        wp_sb[:], wp.rearrange("(l p) n -> p l n", p=P)
                    )
                    pair_tp(2, 0, 8)
                    emit_b_pass(7, 0, lambda jc: 44)
                    flush_bursts()
                    pair_tp(3, 0, 8)
                    filler.extend(
                        lambda t=t, o=o: c_tile(t, o)
                        for t in range(8) for o in range(2)
                    )
                    emit_b_pass(2, 1, pp1_pace(2))
                    emit_b_pass(3, 1, pp1_pace(3))
                    emit_b_pass(4, 1, pp1_pace(4))
                    pair_tp(1, 8, NJC)
                    emit_b_pass(5, 1, pp1_pace(5))
                    emit_b_pass(6, 1, pp1_pace(6))
                    pair_tp(2, 8, NJC)

                    def tail(qc):
                        # (6,1) normalized qc-3..qc already; once (7,1)
                        # normalizes them too, flip pair 3 and project
                        while filler:
                            filler.popleft()()
                        pair_tp(3, qc - 3, qc + 1)
                        for t in range(qc - 3, qc + 1):
                            for o in range(2):
                                c_tile(t, o)

                    emit_b_pass(7, 1, pp1_pace(7), post_norm=tail)
                    flush_bursts()
                    while filler:
                        filler.popleft()()


_CACHE = {}


def build_nc():
    if "nc" in _CACHE:
        return _CACHE["nc"]
    nc = bacc.Bacc(
        "TRN2",
        target_bir_lowering=False,
        debug=False,
        enable_asserts=False,
        num_devices=8,
    )
    aps = {
        "wqk_hi": nc.dram_tensor("wqk_hi", [JQK, P, KC, P], fp8, kind="ExternalInput").ap(),
        "wqk_lo": nc.dram_tensor("wqk_lo", [JQK, P, KC, P], fp8, kind="ExternalInput").ap(),
        "bqk": nc.dram_tensor("bqk", [2 * CL], f32, kind="ExternalInput").ap(),
        "wv_hi": nc.dram_tensor("wv_hi", [C, CL], fp8, kind="ExternalInput").ap(),
        "wv_lo": nc.dram_tensor("wv_lo", [C, CL], fp8, kind="ExternalInput").ap(),
        "x_hi": nc.dram_tensor("x_hi", [C, T], fp8, kind="ExternalInput").ap(),
        "x_lo": nc.dram_tensor("x_lo", [C, T], fp8, kind="ExternalInput").ap(),
        "bv": nc.dram_tensor("bv", [CL], f32, kind="ExternalInput").ap(),
        "wp": nc.dram_tensor("wp", [CL, C], bf16, kind="ExternalInput").ap(),
        "masks": nc.dram_tensor("masks", [P, P], bf16, kind="ExternalInput").ap(),
        "outp": nc.dram_tensor("outp", [T, C], bf16, kind="ExternalOutput").ap(),
    }
    global wp
    wp = aps["wp"]
    with tile.TileContext(nc) as tc:
        build_body(tc, aps)
    nc.compile()
    _CACHE["nc"] = nc
    return nc


def make_masks():
    return np.triu(np.ones((P, P), dtype=np.float32)).astype(ml_dtypes.bfloat16)


def _split8(a, scale):
    f8 = ml_dtypes.float8_e4m3
    hi = (a * scale).astype(f8)
    lo = (a * scale - hi.astype(np.float32)).astype(f8)
    return hi, lo


def make_in_maps(x, w_attn, b_attn, w_proj, b_proj):
    masks = make_masks()
    in_maps = []
    xTs = [np.ascontiguousarray(x[b].T) for b in range(B)]
    xsplits = [_split8(xT, 1.0) for xT in xTs]
    for core in range(8):
        b, g = core // 2, core % 2
        qcols = slice(g * CL, (g + 1) * CL)
        kcols = slice(C + g * CL, C + (g + 1) * CL)
        vcols = slice(2 * C + g * CL, 2 * C + (g + 1) * CL)
        wqk = np.concatenate(
            [w_attn[:, qcols], w_attn[:, kcols]], axis=1
        ).astype(np.float32)
        wqk_hi, wqk_lo = _split8(wqk, SW)
        resh = lambda a: np.ascontiguousarray(
            a.reshape(KC, P, JQK, P).transpose(2, 1, 0, 3)
        )
        wv_hi, wv_lo = _split8(w_attn[:, vcols].astype(np.float32), SV)
        x_hi, x_lo = xsplits[b]
        in_maps.append(
            {
                "wqk_hi": resh(wqk_hi),
                "wqk_lo": resh(wqk_lo),
                "bqk": np.ascontiguousarray(
                    np.concatenate([b_attn[qcols], b_attn[kcols]]) * SW
                ),
                "wv_hi": np.ascontiguousarray(wv_hi),
                "wv_lo": np.ascontiguousarray(wv_lo),
                "x_hi": x_hi,
                "x_lo": x_lo,
                "bv": np.ascontiguousarray(b_attn[vcols] * SV),
                "wp": np.ascontiguousarray(
                    w_proj[g * CL : (g + 1) * CL, :].astype(ml_dtypes.bfloat16)
                ),
                "masks": masks,
            }
        )
    return in_maps


def combine(parts, b_proj):
    return np.stack(
        [
            parts[2 * b].astype(np.float32)
            + parts[2 * b + 1].astype(np.float32)
            + b_proj[None, :]
            for b in range(B)
        ]
    ).astype(np.float32)


def kernel(x, w_attn, b_attn, w_proj, b_proj, _trace=False, **run_kwargs):
    x = np.asarray(x, dtype=np.float32)
    w_attn = np.asarray(w_attn, dtype=np.float32)
    b_attn = np.asarray(b_attn, dtype=np.float32)
    w_proj = np.asarray(w_proj, dtype=np.float32)
    b_proj = np.asarray(b_proj, dtype=np.float32)

    nc = build_nc()
    in_maps = make_in_maps(x, w_attn, b_attn, w_proj, b_proj)
    try:
        res = bass_utils.run_bass_kernel_spmd(
            nc, in_maps, core_ids=list(range(8)), trace=_trace, **run_kwargs
        )
    except Exception:
        res = bass_utils.run_bass_kernel_spmd(
            nc, in_maps, core_ids=list(range(8)), trace=_trace, **run_kwargs
        )
    parts = [res.results[i]["outp"] for i in range(8)]
    out = combine(parts, b_proj)
    if _trace:
        return out, res
    return out



# revision 68
# speedup vs baseline: 1.0379x; 1.0379x over previous
"""Causal self-attention (B=4, T=2048, C=1024, H=16, D=64) on 8 trn2 cores.

Sharding: core i handles batch b = i//2 and head-group g = i%2 (8 of 16
heads), tensor-parallel over c_attn columns / c_proj rows. Each core
computes qkv for its heads, causal attention, and a partial projection
(its 512 rows of w_proj); the host sums the two partials per batch and
adds b_proj.

v2: qkv-gen uses fp8e4 DoubleRow matmuls (0.5 cyc/col, 2 K-tiles per
instruction) with host-precomputed hi/lo fp8 splits of x (scale 1) and
w (scale 64 qk / 32 v); three passes (wh*xh + wl*xh + wh*xl) recover
bf16-grade accuracy at 6/8 the bf16 matmul cost. The attention S matmul
is one fp8 DoubleRow op per strip: the stationary packs (k_hi, k_lo) as
the two K-tiles (k exact via the tile-sum), the moving operand
broadcasts a single-fp8 q; the k path skips its x_lo correction pass
(measured output rel err 1.63e-2; gate 2e-2).
k biases ride in the k_hi cast (zero for these inputs). es/vaug/yT/proj run bf16 (exp scale
0.125/4096, no f32r widening), output is written bf16 and summed on the
host. psum->sbuf q8/k casts sit on DVE to keep ACT (exp-bound) clear.

v3: the A.V matmul is reoriented: out[128 tok, D+1] = es(stationary)^T
x vaug(moving) so the modeled cost is 65 cols/instr instead of the
q-strip width (PE A.V 58us -> 29us). PSUM zero-region rules allow only
one open accumulation group per bank, so A.V runs qc-major (a burst of
jc<=qc matmuls per q-chunk) with every strip's es kept live in a
[P,16,1024] tile, ping-ponged across (h,pp) groups. Softmax
denominators land per-token-partition: normalization is a [P,4]
reciprocal + one broadcast tensor_tensor per 4-chunk group. y[tok,feat]
flips to yT[feat,tok] via XBAR dma_start_transpose (DMA-resource only,
no engine time).

v4 (220us -> 186us total): engine rebalance + scheduling. Three
full-width strips per (h,1) pass compute exp on DVE instead of ACT via
a Schraudolph int16 bit-trick (bf16 bits = trunc(psum*a+b), ~1.8% rms
on es, +0.02e-2 on output since softmax num/den errors cancel); the
diag-mask multiply is SBUF-only so it runs on the idle GPSIMD engine.
A.V bursts trail their strip by 4 via a thunk queue (es ping-pong
makes that legal), A-phase chunk thunks drip through a paced filler
deque tuned so the PE-crunched front stays fed while tci-2/3 qk chunks
defer into the ACT-bound pp1 stretch, and the final c_tiles stagger
into (7,1) via a post-norm hook (tail 17us -> 3us). 14 dummy matmuls
at t~1us warm the PE p-state ramp before the first real chunk lands.
Engine busy: PE 144us (binding), ACT 125us, DVE 116us.
"""

import sys

sys.path.insert(0, "/opt/trn_rl_repo")

from collections import deque
from contextlib import ExitStack

import ml_dtypes
import numpy as np

import concourse.bass as bass
import concourse.mybir as mybir
import concourse.tile as tile
from concourse import bacc
from concourse import bass_utils

f32 = mybir.dt.float32
f32r = mybir.dt.float32r
bf16 = mybir.dt.bfloat16
fp8 = mybir.dt.float8e4
EXP = mybir.ActivationFunctionType.Exp
MUL = mybir.AluOpType.mult
ADD = mybir.AluOpType.add
DR = mybir.MatmulPerfMode.DoubleRow

B, T, C, H, D = 4, 2048, 1024, 16, 64
HL = H // 2          # 8 heads per core
CL = HL * D          # 512 local feature width
P = 128
KC = C // P          # 8 contraction chunks over C
NJC = T // P         # 16 token chunks of 128
NTC = T // 512       # 4 t-chunks of 512 in phase A
JQK = 2 * CL // P    # 8 qk feature chunks of 128

SW = 64.0            # host scale on w_qk (psum = 64*qk)
SV = 32.0            # host scale on w_v (v path scaled by 32)
EXP_SCALE = 0.125 / (SW * SW)

# position of v_chunk[j] in the filler deque (q-chunk leftovers interleave)
POS_V = [0, 1, 3, 4, 6, 7, 9, 10, 12, 13, 14, 15, 16, 17, 18, 19]

# Schraudolph-style exp on DVE: bf16 bits of exp(psum*EXP_SCALE) ~=
# trunc(psum*SCH_A + SCH_B) as int16 (rel err ~1.8% rms, zero-mean).
# Offloads full-width strips from the ACT engine (the busiest engine).
SCH_A = (2.0**23 / np.log(2.0)) / 65536.0 * EXP_SCALE
SCH_B = 16249.0
SCH_JC = (2, 4, 6)   # which pp=1 full-width strips go to DVE


def build_body(tc, aps):
    nc = tc.nc
    bqk, bv, wp, masks, outp = (
        aps["bqk"], aps["bv"], aps["wp"], aps["masks"], aps["outp"],
    )

    with ExitStack() as ctx:
        const = ctx.enter_context(tc.tile_pool(name="const", bufs=1))
        qkT_pool = ctx.enter_context(tc.tile_pool(name="qkT", bufs=1))
        vaug_pool = ctx.enter_context(tc.tile_pool(name="vaug", bufs=1))
        yT_pool = ctx.enter_context(tc.tile_pool(name="yT", bufs=1))

        # const tiles; their DMAs are emitted after the critical xh/xl/wq
        # ones (the DMA resource is serial and these aren't needed early)
        masks_sb = const.tile([P, P], bf16)
        bqk_sb = const.tile([P, JQK], f32)
        bv_rep = const.tile([P, CL], f32)

        def const_dmas():
            nc.sync.dma_start(bqk_sb[:], bqk.rearrange("(j p) -> p j", p=P))
            nc.sync.dma_start(masks_sb[:], masks[:])
            nc.sync.dma_start(bv_rep[:], bv[None, :].to_broadcast([P, CL]))

        q8 = qkT_pool.tile([P, JQK // 2, T], fp8, name="q8")
        khl = qkT_pool.tile([P, JQK // 2, 2, T], fp8, name="khl")
        vaug = vaug_pool.tile([P, NJC, HL, D + 1], bf16)
        # ones column holds the v-path scale so y = num/(SV*sum_es) * SV
        nc.vector.memset(
            vaug[:, :, :, D : D + 1].bitcast(mybir.dt.uint16), 0x4200
        )
        # y in [token-partition, feature] layout; transposed to yT by DMA
        y_sb = yT_pool.tile([P, NJC, HL, D], bf16, name="y_sb")
        # all strips of one (h, pp) group stay live (A.V is qc-major so
        # each psum accumulation group opens/closes sequentially per bank);
        # ping-pong across groups so group g+1's exps don't wait on group
        # g's last A.V burst (which reads every strip slot)
        es_ab = [
            yT_pool.tile([P, NJC, 1024], bf16, name=f"es_all{i}")
            for i in range(2)
        ]
        _gctr = {"g": 0}

        # ---------------- Phase A/B interleaved (same skeleton as v1).
        def qk_chunk_thunks(j, xh_sb, xl_sb):
            wq_box = {}

            def preload():
                wqh_t = wq_pool.tile([P, KC, P], fp8, tag="wqh", name=f"wqh{j}")
                nc.sync.dma_start(wqh_t[:], aps["wqk_hi"][j])
                wql_t = wq_pool.tile([P, KC, P], fp8, tag="wql", name=f"wql{j}")
                nc.sync.dma_start(wql_t[:], aps["wqk_lo"][j])
                wq_box["t"] = (wqh_t, wql_t)

            def load(tci):
                if tci == 0 and "t" not in wq_box:
                    preload()
                wqh_t, wql_t = wq_box["t"]
                is_k = j >= JQK // 2
                ps = psA.tile([P, 512], f32, tag="a")
                cols = slice(tci * 512, tci * 512 + 512)
                # k drops the x_lo correction pass: its ~2.5% error adds in
                # quadrature to the S-stage q8 error (1.19e-2 -> 1.63e-2
                # measured, gate 2e-2) and saves 4 DR matmuls per group.
                # (dropping it on q too was measured at 2.1e-2 — over gate)
                passes = ((wqh_t, xh_sb), (wql_t, xh_sb)) if is_k else (
                    (wqh_t, xh_sb), (wql_t, xh_sb), (wqh_t, xl_sb))
                last = KC // 2 * len(passes) - 1
                n = 0
                # pass-major so the xh-only passes can run before xl lands
                for wt, xt in passes:
                    for i in range(KC // 2):
                        nc.tensor.matmul(
                            ps[:], wt[:, 2 * i : 2 * i + 2, :],
                            xt[:, 2 * i : 2 * i + 2, cols],
                            start=(n == 0),
                            stop=(n == last),
                            perf_mode=DR,
                        )
                        n += 1
                if is_k:
                    jk = j - JQK // 2
                    # split the startup cast so S(0,0,0) only waits on the
                    # first 128 k columns
                    pieces = ((0, P), (P, 512)) if j == 4 and tci == 0 else ((0, 512),)
                    for a, b in pieces:
                        kh = khl[:, jk, 0, tci * 512 + a : tci * 512 + b]
                        nc.vector.tensor_scalar(
                            kh, ps[:, a:b], bqk_sb[:, j : j + 1], None, ADD
                        )
                        nc.vector.tensor_tensor(
                            khl[:, jk, 1, tci * 512 + a : tci * 512 + b],
                            ps[:, a:b], kh,
                            mybir.AluOpType.subtract,
                        )
                else:
                    nc.vector.tensor_scalar(
                        q8[:, j, cols], ps[:], bqk_sb[:, j : j + 1], None, ADD
                    )

            thunks = [lambda tci=tci: load(tci) for tci in range(NTC)]
            return thunks, preload

        def v_chunk_thunks(xh_sb, xl_sb, wvh_sb, wvl_sb):
            def vchunk(tc128):
                ps = psA.tile([P, 512], f32, tag="a")
                cols = slice(tc128 * P, (tc128 + 1) * P)
                n = 0
                for i in range(KC // 2):
                    for xt, wt in (
                        (xh_sb, wvh_sb), (xh_sb, wvl_sb), (xl_sb, wvh_sb),
                    ):
                        nc.tensor.matmul(
                            ps[:], xt[:, 2 * i : 2 * i + 2, cols],
                            wt[:, 2 * i : 2 * i + 2, :],
                            start=(n == 0), stop=(n == 3 * KC // 2 - 1),
                            perf_mode=DR,
                        )
                        n += 1
                nc.vector.tensor_tensor(
                    vaug[:, tc128, :, 0:D],
                    ps[:].rearrange("p (h d) -> p h d", h=HL),
                    bv_rep[:].rearrange("p (h d) -> p h d", h=HL), ADD,
                )

            return [lambda t=t: vchunk(t) for t in range(NJC)]

        def emit_b_pass(h, pp, pace, post_norm=None):
            pr, half = h // 2, h % 2
            base = half * 64

            es_all = es_ab[_gctr["g"] % 2]
            _gctr["g"] += 1
            lo = pp * 1024
            strip_start = {}
            box = {}

            def burst(qc):
                # A.V for q-chunk qc: one psum accumulation group open at a
                # time per bank (zero-region rule). Runs 2 strips late via
                # burst_q so next strips' S matmuls outrank it.
                need(POS_V[qc] + 1)
                s = qc - 8 * pp
                if s % 4 == 0:
                    box["grp"] = psY.tile(
                        [P, 4, D + 1], f32, tag="yt",
                        name=f"yt{h}_{pp}_{s // 4}",
                    )
                grp = box["grp"]
                slot = s % 4
                for j2 in range(qc + 1):
                    st2 = strip_start[j2]
                    nc.tensor.matmul(
                        grp[:, slot, :],
                        es_all[:, j2, qc * P - st2 : qc * P - st2 + P],
                        vaug[:, j2, h, :],
                        start=(j2 == 0), stop=(j2 == qc),
                    )
                if s % 4 == 3:
                    rec4 = nrm_pool.tile([P, 4], f32, tag="rec")
                    nc.vector.reciprocal(
                        rec4[:, :, None], grp[:, :, D : D + 1]
                    )
                    nc.vector.tensor_tensor(
                        y_sb[:, qc - 3 : qc + 1, h, :],
                        grp[:, :, 0:D],
                        rec4[:, :, None].to_broadcast([P, 4, D]),
                        MUL,
                    )
                    if post_norm is not None:
                        post_norm(qc)

            for jc in range(8 * pp + 8):
                need(pace(jc))
                diag = jc * P >= lo
                start = max(jc * P, lo)
                strip_start[jc] = start
                ps_s = psS.tile([P, 1024], f32, tag="s")
                for icp in range(2):
                    ic = 2 * pp + icp
                    if (ic + 1) * 512 <= start:
                        continue
                    c0 = max(start, ic * 512)
                    nc.tensor.matmul(
                        ps_s[:, c0 - lo : (icp + 1) * 512],
                        khl[base : base + 64, pr, :, jc * P : (jc + 1) * P],
                        q8[base : base + 64, pr : pr + 1, c0 : (ic + 1) * 512]
                        .to_broadcast([64, 2, (ic + 1) * 512 - c0]),
                        start=True, stop=True,
                        perf_mode=DR,
                    )
                w = lo + 1024 - start
                if h == 0 and pp == 0 and jc < 4:
                    # startup: exp each 512-col half as its S matmul lands so
                    # the first exps don't wait for the q8 tci-1 chunk
                    for icp in range(2):
                        c0 = max(start, icp * 512)
                        nc.scalar.activation(
                            es_all[:, jc, c0 - start : (icp + 1) * 512 - start],
                            ps_s[:, c0 - lo : (icp + 1) * 512],
                            EXP, scale=EXP_SCALE,
                        )
                elif pp == 1 and jc in SCH_JC:
                    # DVE exp (Schraudolph int16/bf16 bit trick) to offload
                    # the saturated ACT engine on no-diag full-width strips
                    nc.vector.tensor_scalar(
                        es_all[:, jc, :w].bitcast(mybir.dt.int16),
                        ps_s[:, 0:1024], SCH_A, SCH_B, MUL, ADD,
                    )
                else:
                    nc.scalar.activation(
                        es_all[:, jc, :w], ps_s[:, start - lo : 1024],
                        EXP, scale=EXP_SCALE,
                    )
                if diag:
                    # sbuf-only multiply: runs on the otherwise-idle GPSIMD
                    # engine to keep DVE clear for psum-coupled work
                    nc.gpsimd.tensor_tensor(
                        es_all[:, jc, 0:P], es_all[:, jc, 0:P], masks_sb[:], MUL
                    )
                if jc >= 8 * pp:
                    burst_q.append(lambda qc=jc: burst(qc))
                # pp1 strips 0..7 push no bursts (their q-chunks start at
                # jc=8): drain the carried-over queue 1/strip through that
                # zone so PE isn't left with bare S matmuls there
                pump_bursts(min(4, 7 - jc) if (pp == 1 and jc < 8) else 4)

        def pair_tp(hp, qlo, qhi):
            # yT[feat, tok] <- y_sb[tok, feat] for head pair hp via XBAR
            for qc in range(qlo, qhi):
                nc.sync.dma_start_transpose(
                    yT[:, hp, qc * P : (qc + 1) * P],
                    y_sb[:, qc, 2 * hp : 2 * hp + 2, :],
                )

        yT = yT_pool.tile([P, CL // P, T], bf16)
        with ExitStack() as actx:
            xt_pool = actx.enter_context(tc.tile_pool(name="xt", bufs=1))
            wq_pool = actx.enter_context(tc.tile_pool(name="wq", bufs=6))
            psA = actx.enter_context(tc.tile_pool(name="psA", bufs=3, space="PSUM"))

            filler = deque()
            _dr = {"done": 0}

            def need(k):
                while filler and _dr["done"] < k:
                    filler.popleft()()
                    _dr["done"] += 1

            burst_q = deque()

            def pump_bursts(keep=2):
                while len(burst_q) > keep:
                    burst_q.popleft()()

            def flush_bursts():
                pump_bursts(0)

            with tc.tile_pool(name="wv", bufs=1) as wv_pool:
                xh_sb = xt_pool.tile([P, KC, T], fp8, name="xh")
                xl_sb = xt_pool.tile([P, KC, T], fp8, name="xl")
                xh_r = aps["x_hi"].rearrange("(k p) t -> p k t", p=P)
                xl_r = aps["x_lo"].rearrange("(k p) t -> p k t", p=P)

                def xq_dma(q):
                    cols = slice(q * 512, (q + 1) * 512)
                    nc.sync.dma_start(xh_sb[:, :, cols], xh_r[:, :, cols])
                    nc.sync.dma_start(xl_sb[:, :, cols], xl_r[:, :, cols])

                wvh_sb = wv_pool.tile([P, KC, CL], fp8, name="wvh")
                wvl_sb = wv_pool.tile([P, KC, CL], fp8, name="wvl")

                # warm the PE p-state clock so the first real matmuls run at
                # full speed (the cost model ramps over 3us of continuous
                # execution; PE would otherwise sit cold during the input DMAs)
                nc.vector.memset(es_ab[0][:, 0, 0:512].bitcast(mybir.dt.uint16), 0)
                warm_ps = psA.tile([P, 512], f32, tag="a", name="warm")
                for _ in range(14):
                    nc.tensor.matmul(
                        warm_ps[:], es_ab[0][:, 0, 0:P], es_ab[0][:, 0, 0:512],
                        start=True, stop=True,
                    )

                xq_dma(0)
                q0, pre0 = qk_chunk_thunks(0, xh_sb, xl_sb)
                q4, pre4 = qk_chunk_thunks(4, xh_sb, xl_sb)
                pre0()
                pre4()
                const_dmas()
                q0[0]()
                q4[0]()
                xq_dma(1)
                q0[1]()
                q4[1]()
                nc.sync.dma_start(
                    wvh_sb[:], aps["wv_hi"].rearrange("(k p) n -> p k n", p=P)
                )
                nc.sync.dma_start(
                    wvl_sb[:], aps["wv_lo"].rearrange("(k p) n -> p k n", p=P)
                )
                xq_dma(2)
                xq_dma(3)
                # tci 2,3 of j0/j4 drip through the filler so (0,0)'s strips
                # overlap them instead of serializing the whole eager prefix
                vs = v_chunk_thunks(xh_sb, xl_sb, wvh_sb, wvl_sb)
                filler.extend([
                    vs[0], vs[1], q0[2], vs[2], vs[3], q4[2],
                    vs[4], vs[5], q0[3], vs[6], vs[7], q4[3],
                ])
                filler.extend(vs[8:])
                jt = {j: qk_chunk_thunks(j, xh_sb, xl_sb)[0]
                      for j in (1, 5, 2, 6, 3, 7)}
                filler.extend([jt[1][0], jt[5][0], jt[1][1], jt[5][1]])

                with ExitStack() as bctx:
                    nrm_pool = bctx.enter_context(tc.tile_pool(name="nrm", bufs=1))
                    ostg = bctx.enter_context(tc.tile_pool(name="ostg", bufs=4))
                    wp_pool = bctx.enter_context(tc.tile_pool(name="wp", bufs=1))
                    psS = bctx.enter_context(
                        tc.tile_pool(name="psS", bufs=2, space="PSUM")
                    )
                    psY = bctx.enter_context(
                        tc.tile_pool(name="psY", bufs=1, space="PSUM")
                    )

                    def c_tile(tcb, oc):
                        ps = psA.tile([P, 512], f32, tag="a")
                        for li in range(4):
                            nc.tensor.matmul(
                                ps[:],
                                yT[:, li, tcb * P : (tcb + 1) * P],
                                wp_sb[:, li, oc * 512 : oc * 512 + 512],
                                start=(li == 0), stop=(li == 3),
                            )
                        ot = ostg.tile([P, 512], bf16, tag="o")
                        if tcb >= 8 and (tcb + oc) % 2 == 0:
                            # tail only: ACT is idle there; alternating with
                            # DVE pipelines the copy stage two wide
                            nc.scalar.copy(ot[:], ps[:])
                        else:
                            nc.vector.tensor_copy(ot[:], ps[:])
                        nc.sync.dma_start(
                            outp[tcb * P : (tcb + 1) * P,
                                 oc * 512 : oc * 512 + 512],
                            ot[:],
                        )

                    emit_b_pass(0, 0, lambda jc: 2 + jc)
                    # tci 0,1 of later pairs first; tci 2,3 (only needed by
                    # the pp=1 passes) deferred into the pp1 stretch where
                    # PE otherwise idles behind ACT
                    filler.extend([jt[2][0], jt[6][0], jt[2][1], jt[6][1]])
                    filler.extend([jt[3][0], jt[7][0], jt[3][1], jt[7][1]])
                    filler.extend([jt[1][2], jt[5][2], jt[1][3], jt[5][3]])
                    filler.extend([jt[2][2], jt[6][2], jt[2][3], jt[6][3]])
                    filler.extend([jt[3][2], jt[7][2], jt[3][3], jt[7][3]])
                    emit_b_pass(0, 1, lambda jc: 9 if jc < 10 else jc + 3)
                    emit_b_pass(1, 0, lambda jc: 20 + jc // 2)
                    emit_b_pass(1, 1, lambda jc: min(28, 24 + jc // 3))
                    # each pair_tp is placed 1+ passes after the last writer
                    # pass so its bursts have drained through burst_q without
                    # a flush (a flush lumps A.V work ahead of the next S's)
                    pair_tp(0, 0, 8)
                    emit_b_pass(2, 0, lambda jc: 28)
                    pair_tp(0, 8, NJC)
                    emit_b_pass(3, 0, lambda jc: 28 + (jc + 1) // 2)
                    emit_b_pass(4, 0, lambda jc: 32 + jc // 4)
                    pair_tp(1, 0, 8)
                    emit_b_pass(5, 0, lambda jc: 33 + jc // 4)
                    emit_b_pass(6, 0, lambda jc: 36)
                    wp_sb = wp_pool.tile([P, 4, C], bf16, name="wp_sb")
                    nc.sync.dma_start(
                        wp_sb[:], wp.rearrange("(l p) n -> p l n", p=P)
                    )
                    pair_tp(2, 0, 8)
                    emit_b_pass(7, 0, lambda jc: 36)
                    flush_bursts()
                    pair_tp(3, 0, 8)
                    filler.extend(
                        lambda t=t, o=o: c_tile(t, o)
                        for t in range(8) for o in range(2)
                    )
                    emit_b_pass(2, 1, lambda jc: 36 + jc // 2)
                    emit_b_pass(3, 1, lambda jc: 40 + jc // 2)
                    emit_b_pass(4, 1, lambda jc: 44 + jc // 2)
                    pair_tp(1, 8, NJC)
                    emit_b_pass(5, 1, lambda jc: 50 + jc // 2)
                    emit_b_pass(6, 1, lambda jc: min(60, 56 + jc // 2))
                    pair_tp(2, 8, NJC)

                    def tail(qc):
                        # (6,1) normalized qc-3..qc already; once (7,1)
                        # normalizes them too, flip pair 3 and project
                        while filler:
                            filler.popleft()()
                        pair_tp(3, qc - 3, qc + 1)
                        for t in range(qc - 3, qc + 1):
                            for o in range(2):
                                c_tile(t, o)

                    emit_b_pass(7, 1, lambda jc: 60, post_norm=tail)
                    flush_bursts()
                    while filler:
                        filler.popleft()()


_CACHE = {}


def build_nc():
    if "nc" in _CACHE:
        return _CACHE["nc"]
    nc = bacc.Bacc(
        "TRN2",
        target_bir_lowering=False,
        debug=False,
        enable_asserts=False,
        num_devices=8,
    )
    aps = {
        "wqk_hi": nc.dram_tensor("wqk_hi", [JQK, P, KC, P], fp8, kind="ExternalInput").ap(),
        "wqk_lo": nc.dram_tensor("wqk_lo", [JQK, P, KC, P], fp8, kind="ExternalInput").ap(),
        "bqk": nc.dram_tensor("bqk", [2 * CL], f32, kind="ExternalInput").ap(),
        "wv_hi": nc.dram_tensor("wv_hi", [C, CL], fp8, kind="ExternalInput").ap(),
        "wv_lo": nc.dram_tensor("wv_lo", [C, CL], fp8, kind="ExternalInput").ap(),
        "x_hi": nc.dram_tensor("x_hi", [C, T], fp8, kind="ExternalInput").ap(),
        "x_lo": nc.dram_tensor("x_lo", [C, T], fp8, kind="ExternalInput").ap(),
        "bv": nc.dram_tensor("bv", [CL], f32, kind="ExternalInput").ap(),
        "wp": nc.dram_tensor("wp", [CL, C], bf16, kind="ExternalInput").ap(),
        "masks": nc.dram_tensor("masks", [P, P], bf16, kind="ExternalInput").ap(),
        "outp": nc.dram_tensor("outp", [T, C], bf16, kind="ExternalOutput").ap(),
    }
    global wp
    wp = aps["wp"]
    with tile.TileContext(nc) as tc:
        build_body(tc, aps)
    nc.compile()
    _CACHE["nc"] = nc
    return nc


def make_masks():
    return np.triu(np.ones((P, P), dtype=np.float32)).astype(ml_dtypes.bfloat16)


def _split8(a, scale):
    f8 = ml_dtypes.float8_e4m3
    hi = (a * scale).astype(f8)
    lo = (a * scale - hi.astype(np.float32)).astype(f8)
    return hi, lo


def make_in_maps(x, w_attn, b_attn, w_proj, b_proj):
    masks = make_masks()
    in_maps = []
    xTs = [np.ascontiguousarray(x[b].T) for b in range(B)]
    xsplits = [_split8(xT, 1.0) for xT in xTs]
    for core in range(8):
        b, g = core // 2, core % 2
        qcols = slice(g * CL, (g + 1) * CL)
        kcols = slice(C + g * CL, C + (g + 1) * CL)
        vcols = slice(2 * C + g * CL, 2 * C + (g + 1) * CL)
        wqk = np.concatenate(
            [w_attn[:, qcols], w_attn[:, kcols]], axis=1
        ).astype(np.float32)
        wqk_hi, wqk_lo = _split8(wqk, SW)
        resh = lambda a: np.ascontiguousarray(
            a.reshape(KC, P, JQK, P).transpose(2, 1, 0, 3)
        )
        wv_hi, wv_lo = _split8(w_attn[:, vcols].astype(np.float32), SV)
        x_hi, x_lo = xsplits[b]
        in_maps.append(
            {
                "wqk_hi": resh(wqk_hi),
                "wqk_lo": resh(wqk_lo),
                "bqk": np.ascontiguousarray(
                    np.concatenate([b_attn[qcols], b_attn[kcols]]) * SW
                ),
                "wv_hi": np.ascontiguousarray(wv_hi),
                "wv_lo": np.ascontiguousarray(wv_lo),
                "x_hi": x_hi,
                "x_lo": x_lo,
                "bv": np.ascontiguousarray(b_attn[vcols] * SV),
                "wp": np.ascontiguousarray(
                    w_proj[g * CL : (g + 1) * CL, :].astype(ml_dtypes.bfloat16)
                ),
                "masks": masks,
            }
        )
    return in_maps


def combine(parts, b_proj):
    return np.stack(
        [
            parts[2 * b].astype(np.float32)
            + parts[2 * b + 1].astype(np.float32)
            + b_proj[None, :]
            for b in range(B)
        ]
    ).astype(np.float32)


def kernel(x, w_attn, b_attn, w_proj, b_proj, _trace=False, **run_kwargs):
    x = np.asarray(x, dtype=np.float32)
    w_attn = np.asarray(w_attn, dtype=np.float32)
    b_attn = np.asarray(b_attn, dtype=np.float32)
    w_proj = np.asarray(w_proj, dtype=np.float32)
    b_proj = np.asarray(b_proj, dtype=np.float32)

    nc = build_nc()
    in_maps = make_in_maps(x, w_attn, b_attn, w_proj, b_proj)
    try:
        res = bass_utils.run_bass_kernel_spmd(
            nc, in_maps, core_ids=list(range(8)), trace=_trace, **run_kwargs
        )
    except Exception:
        res = bass_utils.run_bass_kernel_spmd(
            nc, in_maps, core_ids=list(range(8)), trace=_trace, **run_kwargs
        )
    parts = [res.results[i]["outp"] for i in range(8)]
    out = combine(parts, b_proj)
    if _trace:
        return out, res
    return out

